# revision 5
# baseline (speedup 1.0000x reference)
"""CMC memory-bank kernel (scatter_memory) for 8 Trainium2 NeuronCores.

Strategy (replicate banks, shard batch):
  - The host interleaves the two memory banks into one [N, 2, D] table so
    that ONE 1KB indirect-DMA descriptor fetches the l-row AND ab-row for
    a given index together (hardware emits one descriptor per dest
    partition row, reading dest-free-size contiguous bytes at
    idx[p]*row_bytes).
  - Each core holds a full replica of the interleaved table plus a 1/8
    row-slice of each original bank for the output copy.
  - Batch rows are sharded 16 per core.  Partition p = b*8 + c owns
    k-chunk c (512 k's) of local batch row b.  The per-core 16*4096 main
    gathers are issued as 512 indirect DMAs of 128 rows each, landing in
    big staging tiles (8 gathers per tile) so the vector engine does one
    multiply + one reduce per 2048-row group.
  - The momentum scatter-update is computed for all 128 batch rows on
    every core from the ORIGINAL bank rows; each core scatter-writes only
    rows in its own bank slice (host passes masked offsets; out-of-slice
    rows point at an OOB index that the bounds-checked indirect DMA
    skips).  With M=0.5 the update is s = mem[y] + x, out = s/||s||
    (power-of-two scaling is lossless, so the 0.5 factors cancel).
  - The NCE normalization constants Z (global means of exp(logits/T)) and
    the final division are applied on the host during unsharding (tiny).
"""

import json
import math

import numpy as np

# ---------------------------------------------------------------------------
# Problem constants (hardcoded per the harness contract).
B = 128
D = 128
N = 600000
K = 4096  # idx has K+1 columns; [0..4095] main + [4096] remainder
T = 0.07
NCORES = 8
BLOC = B // NCORES  # 16 batch rows per core
SLICE = N // NCORES  # 75000 bank rows per core
OOB_ROW = 1 << 20  # scatter offsets >= SLICE are skipped by bounds check


# ---------------------------------------------------------------------------
# Walrus in this container encodes at most ONE semaphore wait per
# instruction ("Too many sync wait commands").  Tile emits multi-wait
# instructions freely, so split every multi-wait instruction in the BIR
# JSON into single-wait NoOp prefixes on the same engine (same per-engine
# stream position => identical semantics).
def _split_multiwait_bir(bir_json: bytes) -> bytes:
    j = json.loads(bir_json)
    ctr = 0
    changed = False
    for f in j.get("functions", []):
        for bb in f.get("blocks", []):
            insts = bb.get("instructions", [])
            new_insts = None
            for i, inst in enumerate(insts):
                si = inst.get("sync_info") or {}
                ow = si.get("on_wait") or []
                if len(ow) > 1:
                    if new_insts is None:
                        new_insts = insts[:i]
                    for w in ow[:-1]:
                        ctr += 1
                        new_insts.append(
                            {
                                "debug": inst.get("debug", 0),
                                "engine": inst["engine"],
                                "ins": [],
                                "outs": [],
                                "name": f"I-waitsplit-{ctr}",
                                "opcode": "NoOp",
                                "sync_info": {"on_wait": [w], "on_update": []},
                            }
                        )
                    si = dict(si)
                    si["on_wait"] = [ow[-1]]
                    inst = dict(inst)
                    inst["sync_info"] = si
                    changed = True
                if new_insts is not None:
                    new_insts.append(inst)
            if new_insts is not None:
                bb["instructions"] = new_insts
    if not changed:
        return bir_json
    return json.dumps(j).encode()


_shim_installed = False


def _install_compile_shim():
    global _shim_installed
    if _shim_installed:
        return
    import concourse.bass_utils as bass_utils
    import concourse.bass2jax as bass2jax

    orig = bass_utils.compile_bir_kernel

    def patched(bir_json: bytes, tmpdir, neff_name="kernel.neff", *a, **k):
        return orig(_split_multiwait_bir(bir_json), tmpdir, neff_name, *a, **k)

    bass_utils.compile_bir_kernel = patched
    bass2jax.compile_bir_kernel = patched
    _shim_installed = True


# ---------------------------------------------------------------------------
# Device program (SPMD: identical on all cores; all per-core differences
# are carried by the input data).
def build_nc(
    n=N,
    bloc=BLOC,
    kmain=K,
    nslice=SLICE,
    d=D,
    group=8,  # indirect gathers (128 rows each) per staging tile
    copy_chunks=8,
    inv_t=1.0 / T,
    reps=1,  # >1: repeat the whole body (differential timing only)
):
    import concourse.bass as bass
    import concourse.mybir as mybir
    from concourse.tile import TileContext, add_dep_helper

    p = 128
    d2 = 2 * d
    cpb = p // bloc  # partitions per batch row (8)
    kpp = kmain // cpb  # k's per partition (512) == gathers per core / p
    ng = kpp // group  # staging-tile groups
    assert kpp % group == 0

    nc = bass.Bass()
    f32 = mybir.dt.float32
    i32 = mybir.dt.int32

    mem_cat = nc.dram_tensor("mem_cat", [n, 2, d], f32, kind="ExternalInput")
    mem_slice = {
        "l": nc.dram_tensor("mem_l_slice", [nslice, d], f32, kind="ExternalInput"),
        "ab": nc.dram_tensor("mem_ab_slice", [nslice, d], f32, kind="ExternalInput"),
    }
    idxp = nc.dram_tensor("idxp", [p, kpp], i32, kind="ExternalInput")
    idx_rem = nc.dram_tensor("idx_rem", [bloc, 1], i32, kind="ExternalInput")
    # per-partition [ab_rep | l_rep]: slot 0 pairs bank l with ab features
    xcat = nc.dram_tensor("xcat", [p, d2], f32, kind="ExternalInput")
    xcat_loc = nc.dram_tensor("xcat_loc", [bloc, d2], f32, kind="ExternalInput")
    xall_cat = nc.dram_tensor("xall_cat", [p, d2], f32, kind="ExternalInput")
    ymem = nc.dram_tensor("ymem", [p, 1], i32, kind="ExternalInput")
    yoff = nc.dram_tensor("yoff", [p, 1], i32, kind="ExternalInput")

    out_part = {  # slot 0 (bank l) -> out_ab; slot 1 (bank ab) -> out_l
        0: nc.dram_tensor("out_ab_part", [bloc, kmain + 1], f32, kind="ExternalOutput"),
        1: nc.dram_tensor("out_l_part", [bloc, kmain + 1], f32, kind="ExternalOutput"),
    }
    new_mem = {
        0: nc.dram_tensor("new_mem_l", [nslice, d], f32, kind="ExternalOutput"),
        1: nc.dram_tensor("new_mem_ab", [nslice, d], f32, kind="ExternalOutput"),
    }

    with TileContext(nc) as tc:
        with (
            tc.tile_pool(name="gp", bufs=3) as gp,
            tc.tile_pool(name="pp", bufs=2) as pp,
            tc.tile_pool(name="cols", bufs=2) as colsp,
            tc.tile_pool(name="small", bufs=1) as sp,
        ):
            for rep in range(reps):
                # ---- bank copies (DRAM -> DRAM), chunked; two HWDGE rings
                copy_insts = {0: [], 1: []}
                rows_per_chunk = math.ceil(nslice / copy_chunks)
                for slot, (bank, eng) in enumerate(
                    (("l", nc.sync), ("ab", nc.scalar))
                ):
                    for ci in range(copy_chunks):
                        r0 = ci * rows_per_chunk
                        r1 = min(nslice, r0 + rows_per_chunk)
                        if r0 >= r1:
                            continue
                        di = eng.dma_start(
                            out=new_mem[slot][r0:r1, :],
                            in_=mem_slice[bank][r0:r1, :],
                        )
                        copy_insts[slot].append(di)

                # ---- index tile + x tiles
                idx_sb = sp.tile([p, kpp], i32)
                nc.sync.dma_start(out=idx_sb[:], in_=idxp[:])
                xc = sp.tile([p, d2], f32)
                nc.sync.dma_start(out=xc[:], in_=xcat[:])
                xc4 = xc[:].rearrange("p (one c) -> p one c", one=1).to_broadcast(
                    [p, group, d2]
                )

                # ---- main gathers: group indirect DMAs into one staging tile,
                # then one multiply + one reduce per group
                cols2 = colsp.tile([p, kpp * 2], f32, tag="cols2")
                for gi in range(ng):
                    gbig = gp.tile([p, group * d2], f32, tag="g")
                    for u in range(group):
                        t = gi * group + u
                        nc.gpsimd.indirect_dma_start(
                            out=gbig[:, u * d2 : (u + 1) * d2],
                            out_offset=None,
                            in_=mem_cat[:],
                            in_offset=bass.IndirectOffsetOnAxis(
                                ap=idx_sb[:, t : t + 1], axis=0
                            ),
                        )
                    prod = pp.tile([p, group * d2], f32, tag="prod")
                    nc.vector.tensor_tensor(
                        out=prod[:],
                        in0=gbig[:].rearrange("p (u c) -> p u c", c=d2),
                        in1=xc4,
                        op=mybir.AluOpType.mult,
                    )
                    nc.vector.tensor_reduce(
                        out=cols2[:, gi * group * 2 : (gi + 1) * group * 2],
                        in_=prod[:].rearrange("p (u two d) -> p u two d", two=2, d=d),
                        axis=mybir.AxisListType.X,
                        op=mybir.AluOpType.add,
                    )

                # ---- exp (deinterleave while applying): slot s at stride 2
                for slot in (0, 1):
                    expo = colsp.tile([p, kpp], f32, tag=f"expo_{slot}")
                    nc.scalar.activation(
                        out=expo[:],
                        in_=cols2[:].rearrange("p (t two) -> p t two", two=2)[
                            :, :, slot
                        ],
                        func=mybir.ActivationFunctionType.Exp,
                        scale=float(inv_t),
                    )
                    nc.sync.dma_start(
                        out=out_part[slot][:, 0:kmain].rearrange(
                            "b (c j) -> b c j", c=cpb
                        ),
                        in_=expo[:],
                    )

                # ---- remainder column kmain
                idxr = sp.tile([bloc, 1], i32)
                nc.sync.dma_start(out=idxr[:], in_=idx_rem[:])
                gr = sp.tile([bloc, d2], f32)
                nc.gpsimd.indirect_dma_start(
                    out=gr[:],
                    out_offset=None,
                    in_=mem_cat[:],
                    in_offset=bass.IndirectOffsetOnAxis(ap=idxr[:, :1], axis=0),
                )
                xlc = sp.tile([bloc, d2], f32)
                nc.sync.dma_start(out=xlc[:], in_=xcat_loc[:])
                prodr = sp.tile([bloc, d2], f32)
                nc.vector.tensor_tensor(
                    out=prodr[:], in0=gr[:], in1=xlc[:], op=mybir.AluOpType.mult
                )
                dotr = sp.tile([bloc, 2], f32)
                nc.vector.tensor_reduce(
                    out=dotr[:],
                    in_=prodr[:].rearrange("b (two d) -> b two d", d=d),
                    axis=mybir.AxisListType.X,
                    op=mybir.AluOpType.add,
                )
                expr = sp.tile([bloc, 2], f32)
                nc.scalar.activation(
                    out=expr[:],
                    in_=dotr[:],
                    func=mybir.ActivationFunctionType.Exp,
                    scale=float(inv_t),
                )
                for slot in (0, 1):
                    nc.sync.dma_start(
                        out=out_part[slot][:, kmain : kmain + 1],
                        in_=expr[:, slot : slot + 1],
                    )

                # ---- momentum scatter-update (both banks at once)
                ym = sp.tile([p, 1], i32)
                nc.sync.dma_start(out=ym[:], in_=ymem[:])
                yo = sp.tile([p, 1], i32)
                nc.sync.dma_start(out=yo[:], in_=yoff[:])
                xa = sp.tile([p, d2], f32)
                nc.sync.dma_start(out=xa[:], in_=xall_cat[:])
                gm = sp.tile([p, d2], f32)
                nc.gpsimd.indirect_dma_start(
                    out=gm[:],
                    out_offset=None,
                    in_=mem_cat[:],
                    in_offset=bass.IndirectOffsetOnAxis(ap=ym[:, :1], axis=0),
                )
                s = sp.tile([p, d2], f32)
                nc.vector.tensor_tensor(
                    out=s[:], in0=gm[:], in1=xa[:], op=mybir.AluOpType.add
                )
                ssq = sp.tile([p, d2], f32)
                nc.vector.tensor_tensor(
                    out=ssq[:], in0=s[:], in1=s[:], op=mybir.AluOpType.mult
                )
                sumsq = sp.tile([p, 2], f32)
                nc.vector.tensor_reduce(
                    out=sumsq[:],
                    in_=ssq[:].rearrange("p (two d) -> p two d", d=d),
                    axis=mybir.AxisListType.X,
                    op=mybir.AluOpType.add,
                )
                nrm = sp.tile([p, 2], f32)
                nc.scalar.sqrt(out=nrm[:], in_=sumsq[:])
                rnrm = sp.tile([p, 2], f32)
                nc.vector.reciprocal(out=rnrm[:], in_=nrm[:])
                updn = sp.tile([p, d2], f32)
                nc.vector.tensor_tensor(
                    out=updn[:],
                    in0=s[:].rearrange("p (two d) -> p two d", d=d),
                    in1=rnrm[:]
                    .rearrange("p (two one) -> p two one", one=1)
                    .to_broadcast([p, 2, d]),
                    op=mybir.AluOpType.mult,
                )
                for slot in (0, 1):
                    sc = nc.gpsimd.indirect_dma_start(
                        out=new_mem[slot][:],
                        out_offset=bass.IndirectOffsetOnAxis(ap=yo[:, :1], axis=0),
                        in_=updn[:, slot * d : (slot + 1) * d],
                        in_offset=None,
                        bounds_check=nslice - 1,
                        oob_is_err=False,
                    )
                    # the scatter must land after the bank copy (WAW)
                    for di in copy_insts[slot]:
                        add_dep_helper(sc.ins, di.ins, True)

    return nc


# ---------------------------------------------------------------------------
# Host side: shard, run, unshard.
_cached_nc = None


def _get_nc():
    global _cached_nc
    if _cached_nc is None:
        _install_compile_shim()
        _cached_nc = build_nc()
    return _cached_nc


def _make_in_maps(l, ab, y, idx, memory_l, memory_ab):
    l = np.ascontiguousarray(l, dtype=np.float32)
    ab = np.ascontiguousarray(ab, dtype=np.float32)
    y = np.ascontiguousarray(y).astype(np.int32, copy=False)
    idx = np.ascontiguousarray(idx).astype(np.int32, copy=False)
    memory_l = np.ascontiguousarray(memory_l, dtype=np.float32)
    memory_ab = np.ascontiguousarray(memory_ab, dtype=np.float32)

    mem_cat = np.stack([memory_l, memory_ab], axis=1)  # [N, 2, D]
    xall_cat = np.concatenate([l, ab], axis=1)  # [B, 2D]

    # scatter ownership: row y[b] belongs to core y[b] // SLICE.  For
    # duplicate y values only the LAST occurrence wins (matches jax
    # .at[y].set semantics); earlier dups are redirected to the OOB row.
    yoff_all = np.full((NCORES, B), OOB_ROW, dtype=np.int32)
    seen = set()
    for b in range(B - 1, -1, -1):
        yb = int(y[b])
        if yb in seen:
            continue
        seen.add(yb)
        c = yb // SLICE
        yoff_all[c, b] = yb - c * SLICE

    in_maps = []
    for c in range(NCORES):
        b0 = c * BLOC
        lo = c * SLICE
        idx_c = idx[b0 : b0 + BLOC]
        ab_rep = np.repeat(ab[b0 : b0 + BLOC], B // BLOC, axis=0)
        l_rep = np.repeat(l[b0 : b0 + BLOC], B // BLOC, axis=0)
        in_maps.append(
            {
                "mem_cat": mem_cat,
                "mem_l_slice": memory_l[lo : lo + SLICE],
                "mem_ab_slice": memory_ab[lo : lo + SLICE],
                "idxp": np.ascontiguousarray(idx_c[:, :K].reshape(128, K // 8)),
                "idx_rem": np.ascontiguousarray(idx_c[:, K : K + 1]),
                "xcat": np.concatenate([ab_rep, l_rep], axis=1),
                "xcat_loc": np.concatenate(
                    [ab[b0 : b0 + BLOC], l[b0 : b0 + BLOC]], axis=1
                ),
                "xall_cat": xall_cat,
                "ymem": y[:, None],
                "yoff": yoff_all[c][:, None],
            }
        )
    return in_maps


def kernel(l, ab, y, idx, memory_l, memory_ab):
    from concourse.bass_utils import run_bass_kernel_spmd

    nc = _get_nc()
    in_maps = _make_in_maps(l, ab, y, idx, memory_l, memory_ab)
    res = run_bass_kernel_spmd(nc, in_maps, core_ids=list(range(NCORES)))
    r = res.results

    out_ab = np.concatenate([r[c]["out_ab_part"] for c in range(NCORES)], axis=0)
    out_l = np.concatenate([r[c]["out_l_part"] for c in range(NCORES)], axis=0)
    new_memory_l = np.concatenate([r[c]["new_mem_l"] for c in range(NCORES)], axis=0)
    new_memory_ab = np.concatenate([r[c]["new_mem_ab"] for c in range(NCORES)], axis=0)

    z_l = np.float32(out_l.mean(dtype=np.float64) * N)
    z_ab = np.float32(out_ab.mean(dtype=np.float64) * N)
    out_l = out_l / z_l
    out_ab = out_ab / z_ab
    return out_l, out_ab, new_memory_l, new_memory_ab
